# revision 28
# baseline (speedup 1.0000x reference)
"""LocalIsing energy kernel for Trainium2 (8 NeuronCores, data-parallel over batch).

reference:  energy[b] = x[b] @ J1 + sum_c J2[c] * x[b, p0[c]] * x[b, p1[c]]

The pair term is a quadratic form: scatter-add J2 into W[512,512] at (p0,p1)
(host-side, cheap: 130816 elements), then with the (symmetric) form folded
into a strictly-upper-triangular Wu (Wu = triu(W+W.T,1) + diag(W)):
    energy[b] = sum_j (x @ Wu)[b,j] * x[b,j] + x[b] @ J1
Each core handles 128 batch rows.

v3 computes everything in the TRANSPOSED orientation so the x shard is only
shipped once (as xt[j,b] = x[b,j] k-blocks):
    yT_g[j,b] = sum_k  Wu_kg[i,j]^T  xt_k[i,b]      (PE, bank g of PSUM)
    z_g[j,b]  = yT_g[j,b] * xt_g[j,b]               (DVE, per bank)
    e[b]      = sum_g ( ones[j] @ z_g  +  J1_g[j] @ xt_g )   (PE, rank-1
                 column matmuls accumulate the partition-axis sums into a
                 [1,128] PSUM row)
so no straight-layout copy of x and no 128-row broadcast of J1 is needed;
J1 rides as four [128,1] columns inside the first chunk.  Output is a single
[1,128] f32 row = one 512B DMA descriptor.

The 449KB/core input stream (W upper-tri pack 320KB + xt 128KB + aux 2KB,
all fp16, W/J1 pre-scaled by 2^16 to clear fp16's subnormal range; the final
copy rescales) is cut into seven ~64KB chunks alternated across the two
HWDGE rings (sync + scalar) in consumption order, so the PE chases the
stream chunk-by-chunk; each bank's DVE multiply runs as soon as that bank's
accumulation stops.
"""

import numpy as np
from contextlib import ExitStack

import concourse.tile as tile
from concourse import bacc, mybir
from concourse.bass_utils import run_bass_kernel_spmd

N = 512          # spins
B = 1024         # batch
NCORES = 8
BS = B // NCORES  # 128 rows per core = one partition tile
KT = N // 128     # 4 contraction tiles
AUX = 8          # aux columns in chunk c1: j1c (4) | ones (1) | pad (3)

_cached_nc = None


SC = 2.0 ** 16   # fp16 scale: J-values ~1e-4 sit in fp16's subnormal range;
                 # scaling up (exact power of 2) keeps them normal; the final
                 # PSUM->SBUF copy multiplies by 1/SC.


def _build():
    f32 = mybir.dt.float32
    f16 = mybir.dt.float16
    nc = bacc.Bacc(
        "TRN2", target_bir_lowering=False, debug=False, num_devices=1
    )
    i8 = mybir.dt.int8
    widths = {
        "c1": 128 + AUX + 128, "c2": 768, "d1": 256, "d2": 128,
    }
    drs = {
        nm: nc.dram_tensor(nm, [BS, w], f16, kind="ExternalInput")
        for nm, w in widths.items()
    }
    # xt1..xt3 travel as int8 (+-1 is exact) on the gpsimd SWDGE ring and
    # cast to fp16 in flight — 48KB instead of 96KB off the HWDGE rings
    xt8 = nc.dram_tensor("xt8", [BS, 384], i8, kind="ExternalInput")
    en = nc.dram_tensor("energy", [1, BS], f32, kind="ExternalOutput")

    with tile.TileContext(nc) as tc, ExitStack() as ctx:
        sb = ctx.enter_context(tc.tile_pool(name="sb", bufs=1))
        ps = ctx.enter_context(tc.tile_pool(name="ps", bufs=1, space="PSUM"))

        c1 = sb.tile([BS, widths["c1"]], f16, tag="c1")
        c2 = sb.tile([BS, widths["c2"]], f16, tag="c2")
        d1 = sb.tile([BS, widths["d1"]], f16, tag="d1")
        d2 = sb.tile([BS, widths["d2"]], f16, tag="d2")
        xts = sb.tile([BS, 384], f16, tag="xts")
        # wide chunks (≥1.3KB descriptors), consumption-ordered; ~2.7:1 of
        # the HWDGE bytes ride the scalar (ACT) ring, which empirically
        # drains ~2x faster than the sync (SP) ring that also carries the
        # output DMA; the last-needed W33 sits alone in a tiny tail chunk
        nc.scalar.dma_start(c1, drs["c1"][:, :])
        nc.sync.dma_start(d1, drs["d1"][:, :])
        nc.gpsimd.dma_start(xts, xt8[:, :])
        nc.scalar.dma_start(c2, drs["c2"][:, :])
        nc.sync.dma_start(d2, drs["d2"][:, :])

        # chunk slicing (must match _pack_inputs)
        xt = [c1[:, 0:128], xts[:, 0:128], xts[:, 128:256], xts[:, 256:384]]
        j1c = c1[:, 128:132]          # [128, 4]: column g = SC*J1[128g:...]
        ones = c1[:, 132:133]         # [128, 1] of 1.0
        w = {  # (k, g) -> lhsT = Wu[128k:.., 128g:..] block
            (0, 0): c1[:, 136:264],
            (0, 1): c2[:, 0:128], (0, 2): c2[:, 128:256],
            (0, 3): c2[:, 256:384],
            (1, 1): c2[:, 384:512], (1, 2): c2[:, 512:640],
            (1, 3): c2[:, 640:768],
            (2, 2): d1[:, 0:128], (2, 3): d1[:, 128:256],
            (3, 3): d2[:, 0:128],
        }

        # yT_g[j, b] accumulates W-tiles k<=g; bank g is final after (k=g, g)
        y = [
            ps.tile([BS, 128], f32, name=f"y{g}", tag=f"y{g}") for g in range(KT)
        ]
        for k in range(KT):
            for g in range(k, KT):
                nc.tensor.matmul(
                    y[g], w[(k, g)], xt[k], start=(k == 0), stop=(k == g)
                )

        # z_g = (yT_g + J1_g) * xt_g on DVE — J1[j] is constant per
        # partition here, so it rides the STT's per-partition scalar stage
        # instead of costing rank-1 matmuls; each bank's STT starts as soon
        # as that bank's accumulation stops
        z = sb.tile([BS, N], f16)
        for g in range(KT):
            nc.vector.scalar_tensor_tensor(
                out=z[:, g * 128 : (g + 1) * 128],
                in0=y[g],
                scalar=j1c[:, g : g + 1],
                op0=mybir.AluOpType.add,
                in1=xt[g],
                op1=mybir.AluOpType.mult,
            )
        # partition-axis sum of each z bank via ones-column matmuls, PSUM-
        # accumulated into one [1,128] row (bank g's matmul chases STT_g)
        e_ps = ps.tile([1, BS], f32, name="e_ps", tag="e_ps")
        for g in range(KT):
            nc.tensor.matmul(
                e_ps, ones, z[:, g * 128 : (g + 1) * 128],
                start=(g == 0), stop=(g == KT - 1),
            )

        # e_sb = e_ps / SC, then a single 512B output descriptor
        e_sb = sb.tile([1, BS], f32)
        nc.vector.tensor_scalar_mul(e_sb, e_ps, 1.0 / SC)
        nc.sync.dma_start(en[:, :], e_sb[:, :])
    nc.finalize()
    return nc


def _pack_inputs(x, J1, J2, pairs):
    x = np.asarray(x, dtype=np.float32)
    J1 = np.asarray(J1, dtype=np.float32)
    J2f = np.asarray(J2, dtype=np.float64)
    pairs = np.asarray(pairs)
    f16 = mybir.dt.np(mybir.dt.float16)

    # Scatter-add J2 into W (handles duplicate pairs exactly like the
    # reference's gather-sum), then fold the (symmetric) quadratic form
    # into a strictly-upper-triangular matrix.
    idx = pairs[:, 0].astype(np.int64) * N + pairs[:, 1].astype(np.int64)
    W = np.bincount(idx, weights=J2f, minlength=N * N).reshape(N, N)
    Wu = (np.triu(W + W.T, 1) + np.diag(np.diag(W))) * SC
    wg = {
        (k, g): np.ascontiguousarray(
            Wu[k * 128 : (k + 1) * 128, g * 128 : (g + 1) * 128]
        ).astype(f16)
        for k in range(KT)
        for g in range(k, KT)
    }
    aux = np.zeros((BS, AUX), np.float32)
    aux[:, 0:4] = (J1 * SC).reshape(4, 128).T
    aux[:, 4] = 1.0
    aux = aux.astype(f16)

    in_maps = []
    for c in range(NCORES):
        xb = x[c * BS : (c + 1) * BS]                      # [128, 512]
        xt = [
            np.ascontiguousarray(xb[:, k * 128 : (k + 1) * 128].T).astype(f16)
            for k in range(KT)
        ]
        cat = lambda *parts: np.ascontiguousarray(np.concatenate(parts, axis=1))
        in_maps.append(
            {
                "c1": cat(xt[0], aux, wg[(0, 0)]),
                "c2": cat(
                    wg[(0, 1)], wg[(0, 2)], wg[(0, 3)],
                    wg[(1, 1)], wg[(1, 2)], wg[(1, 3)],
                ),
                "d1": cat(wg[(2, 2)], wg[(2, 3)]),
                "d2": np.ascontiguousarray(wg[(3, 3)]),
                "xt8": np.ascontiguousarray(
                    np.concatenate(
                        [xb[:, k * 128 : (k + 1) * 128].T for k in (1, 2, 3)],
                        axis=1,
                    )
                ).astype(np.int8),
            }
        )
    return in_maps


def kernel(x, J1, J2, pairs):
    global _cached_nc
    if _cached_nc is None:
        _cached_nc = _build()
    in_maps = _pack_inputs(x, J1, J2, pairs)
    res = run_bass_kernel_spmd(_cached_nc, in_maps, core_ids=list(range(NCORES)))
    return np.concatenate([r["energy"].reshape(-1) for r in res.results])
